# revision 17
# baseline (speedup 1.0000x reference)
"""BBox-aware BCE loss kernel for Trainium2 (8 NeuronCores, data parallel).

Math (exact reformulation of the reference):
  loss = softplus(pred) - pred*target = softplus((1-2t)*pred)   for t in {0,1}
  u = 1-2t in {+1 (t=0), -1 (t=1)}
  Su(i,j) = sum of u over the clipped 5x5 window = (#zeros - #ones)
  edge pixel  <=>  window is mixed  <=>  |Su| < V   (V = clipped window size)
  result = (sum(loss) - 0.9*sum(loss*edge)) / N   (equals the reference in
  both branches of its global `cond`: no edges anywhere => sum(loss*edge)=0).

Device pipeline per core (4 samples, 9 overlapping 128-row tiles each so the
5-tap column window never crosses a tile boundary; owned rows exclude the
2-row overlap):
  GpSimd: casting DMAs (f32 HBM -> bf16 SBUF) for pred/target; first
          row-window pair-add v[j] = u[j]+u[j+1]
  VectorE: u = 1-2t; s = p*u; fused (|Su|<V)*loss with accumulated sums
           (main + image-edge strips with their own thresholds)
  ScalarE: g = exp(s); loss = ln(g*own + 1) (own=0 kills overlap rows);
           |Su| via Abs on the PSUM->SBUF copy
  TensorE: column 5-window sum via 3 shifted accumulating band matmuls
           (Su = A@(v(-2) + v(0) + u(+2))); sum(loss) via ones-matmuls
           accumulated in PSUM over all tiles
Host: float64 reduction of per-core partials.
"""

import sys

import numpy as np

sys.path.insert(0, "/opt/trn_rl_repo")

import ml_dtypes

B, H, W = 32, 1024, 1024
NCORES = 8
SPC = B // NCORES  # samples per core
ROWS = SPC * H
N_TOT = float(B * H * W)

# per-sample tiling: (input_row_start, input_rows, owned_lo, owned_hi)
TILES = [(0, 128, 0, 126)]
for t in range(1, 8):
    TILES.append((124 * t, 128, 2, 126))
TILES.append((992, 32, 2, 32))
NT = len(TILES)  # 9
NTILES = SPC * NT  # 36

BF16 = ml_dtypes.bfloat16


def _band(k_rows: int, m_lo: int, m_hi: int) -> np.ndarray:
    a = np.zeros((k_rows, 128), dtype=np.float32)
    for k in range(k_rows):
        for m in range(m_lo, m_hi):
            if abs(k - m) <= 2:
                a[k, m] = 1.0
    return a.astype(BF16)


def _vh_of(tile_idx: int) -> np.ndarray:
    """clipped column-window size per in-tile row."""
    in0, p_in, _, _ = TILES[tile_idx]
    vh = np.full(128, 5.0, dtype=np.float32)
    for k in range(p_in):
        img = in0 + k
        vh[k] = min(img, 2) + min(H - 1 - img, 2) + 1
    return vh


def _owned(tile_idx: int) -> np.ndarray:
    _, _, o0, o1 = TILES[tile_idx]
    m = np.zeros(128, dtype=np.float32)
    m[o0:o1] = 1.0
    return m


def _statics() -> dict[str, np.ndarray]:
    s = {}
    s["a_top"] = _band(128, 0, 126)
    s["a_mid"] = _band(128, 2, 126)
    s["a_last"] = _band(32, 2, 32)
    for nm, ti in (("top", 0), ("mid", 1), ("last", 8)):
        vh = _vh_of(ti)
        own = _owned(ti)
        edge_row = (vh < 5.0) & (own > 0)  # image top/bottom rows (owned)
        s[f"ones_{nm}"] = own.astype(BF16).reshape(128, 1)
        s[f"own_{nm}"] = own.reshape(128, 1).astype(np.float32)
        # Per-partition |Su| thresholds; -1 disables a row (never an edge,
        # contributes 0) so the four accumulation regions are exactly
        # disjoint and overlap rows are excluded everywhere.
        # main: interior cols, vh=5 owned rows only
        thrm = np.where(own > 0, 24.5, -1.0)
        thrm = np.where(edge_row, -1.0, thrm)
        s[f"thrm_{nm}"] = thrm.reshape(128, 1).astype(np.float32)
        # edge cols {0,W-1}: vw=3; {1,W-2}: vw=4 (all owned rows)
        s[f"thra_{nm}"] = np.where(own > 0, vh * 3.0 - 0.5, -1.0).reshape(
            128, 1).astype(np.float32)
        s[f"thrb_{nm}"] = np.where(own > 0, vh * 4.0 - 0.5, -1.0).reshape(
            128, 1).astype(np.float32)
        # image top/bottom rows, interior cols (vw=5)
        s[f"thrr_{nm}"] = np.where(edge_row, vh * 5.0 - 0.5, -1.0).reshape(
            128, 1).astype(np.float32)
    return s


_CACHED = {}


def _split_multi_waits(nc, mybir):
    """This walrus's core_v3 codegen allows only one sem-wait per
    instruction; peel extra waits onto same-engine NOPs placed just before."""
    skip = (mybir.InstEventSemaphore,)
    k = 0
    for fn in nc.m.functions:
        for blk in fn.blocks:
            out = []
            for inst in blk.instructions:
                si = inst.sync_info
                if (si is not None and len(si.on_wait) > 1
                        and not isinstance(inst, skip)):
                    waits = list(si.on_wait)
                    for w in waits[:-1]:
                        k += 1
                        nop = mybir.InstNoOp(name=f"wsplit-{k}", ins=[], outs=[])
                        nop.engine = inst.engine
                        nop.sync_info = mybir.SyncInfo(on_wait=[w], on_update=[])
                        out.append(nop)
                    inst.sync_info = mybir.SyncInfo(
                        on_wait=[waits[-1]], on_update=list(si.on_update))
                out.append(inst)
            blk.instructions = out


def _build_nc():
    import concourse.bass as bass
    import concourse.mybir as mybir
    import concourse.tile as tile

    f32 = mybir.dt.float32
    bf16 = mybir.dt.bfloat16
    Act = mybir.ActivationFunctionType
    Alu = mybir.AluOpType

    nc = bass.Bass("TRN2", target_bir_lowering=False, debug=False,
                   num_devices=NCORES, num_swdge_queues=4)

    pred_d = nc.dram_tensor("pred", [ROWS, W], f32, kind="ExternalInput").ap()
    tgt_d = nc.dram_tensor("target", [ROWS, W], f32, kind="ExternalInput").ap()
    sd = {}
    statics = _statics()
    for nm, arr in statics.items():
        dt = bf16 if arr.dtype == BF16 else f32
        sd[nm] = nc.dram_tensor(nm, list(arr.shape), dt,
                                kind="ExternalInput").ap()
    out_d = nc.dram_tensor("out", [128, 2], f32, kind="ExternalOutput").ap()

    WP = W + 4  # padded width for the 5-tap row window

    with tile.TileContext(nc) as tc:
        with (
            tc.tile_pool(name="sing", bufs=1) as sing,
            tc.tile_pool(name="tb", bufs=4) as tb_pool,
            tc.tile_pool(name="pb", bufs=4) as pb_pool,
            tc.tile_pool(name="g", bufs=3) as g_pool,
            tc.tile_pool(name="loss", bufs=4) as loss_pool,
            tc.tile_pool(name="asu", bufs=3) as asu_pool,
            tc.tile_pool(name="scr", bufs=3) as scr_pool,
            tc.tile_pool(name="ls", bufs=3) as ls_pool,
            tc.tile_pool(name="psum", bufs=4, space="PSUM") as psum_pool,
        ):
            # ---- statics in SBUF ----
            sb = {}
            for nm, arr in statics.items():
                dt = bf16 if arr.dtype == BF16 else f32
                sb[nm] = sing.tile(list(arr.shape), dt, tag=nm, name=nm)
                nc.sync.dma_start(out=sb[nm][:], in_=sd[nm][:])

            def per_tile(t):
                nm = "top" if t == 0 else ("last" if t == NT - 1 else "mid")
                return (sb[f"a_{nm}"], sb[f"own_{nm}"], sb[f"thrm_{nm}"],
                        sb[f"thra_{nm}"], sb[f"thrb_{nm}"], sb[f"thrr_{nm}"])

            # stats columns: [0:36) main le, [40:76) colA, [80:116) colB,
            # [120:128) row strips, [160:196) loss sums
            stats = sing.tile([128, 224], f32, tag="stats")
            nc.vector.memset(stats[:], 0.0)

            # padded ring buffers (pads zeroed once, never rewritten)
            u_bufs = [sing.tile([128, WP], bf16, tag=f"ub{i}", name=f"ub{i}")
                      for i in range(4)]
            v_bufs = [sing.tile([128, WP], bf16, tag=f"vb{i}", name=f"vb{i}")
                      for i in range(4)]
            for bb in u_bufs + v_bufs:
                nc.vector.memset(bb[:, 0:2], 0.0)
                nc.vector.memset(bb[:, W + 2:WP], 0.0)

            idx = 0
            rowidx = 0
            for smp in range(SPC):
                for t in range(NT):
                    in0, p_in, o0, o1 = TILES[t]
                    r0 = smp * H + in0
                    a_sb, own_sb, thrm_sb, thra_sb, thrb_sb, thrr_sb = per_tile(t)

                    # casting DMAs: f32 HBM -> bf16 SBUF
                    tb = tb_pool.tile([128, W], bf16)
                    nc.gpsimd.dma_start(out=tb[0:p_in],
                                        in_=tgt_d[r0:r0 + p_in, :])
                    pb = pb_pool.tile([128, W], bf16)
                    nc.gpsimd.dma_start(out=pb[0:p_in],
                                        in_=pred_d[r0:r0 + p_in, :])

                    # u = 1 - 2t into padded buffer center
                    ub = u_bufs[idx % 4]
                    nc.vector.tensor_scalar(
                        out=ub[0:p_in, 2:2 + W], in0=tb[0:p_in],
                        scalar1=-2.0, scalar2=1.0, op0=Alu.mult, op1=Alu.add)

                    # s = p*u (in place over pb)
                    nc.vector.tensor_mul(out=pb[0:p_in], in0=pb[0:p_in],
                                         in1=ub[0:p_in, 2:2 + W])

                    # loss = ln(exp(s)*own + 1): own=0 zeroes overlap rows
                    g = g_pool.tile([128, W], bf16)
                    nc.scalar.activation(out=g[0:p_in], in_=pb[0:p_in],
                                         func=Act.Exp)
                    loss = loss_pool.tile([128, W], bf16)
                    nc.scalar.activation(out=loss[0:p_in], in_=g[0:p_in],
                                         func=Act.Ln, bias=1.0)
                    # sum(loss) over owned rows: (loss*own) with accumulate
                    ls = ls_pool.tile([128, W], bf16)
                    nc.vector.tensor_scalar(
                        out=ls[0:p_in], in0=loss[0:p_in],
                        scalar1=own_sb[0:p_in, 0:1], scalar2=0.0,
                        op0=Alu.mult, op1=Alu.add,
                        accum_out=stats[0:p_in, 160 + idx:161 + idx])

                    # v[j] = u[j] + u[j+1] (first half of the row 5-window)
                    vb = v_bufs[idx % 4]
                    nc.gpsimd.tensor_add(out=vb[0:p_in, 0:1026],
                                         in0=ub[0:p_in, 0:1026],
                                         in1=ub[0:p_in, 1:1027])

                    # Su = A @ (v(-2) + v(0) + u(+2)): 3 accumulating matmuls
                    sup = psum_pool.tile([128, W], f32)
                    for h in (0, 512):
                        nc.tensor.matmul(sup[:, h:h + 512], a_sb[0:p_in, :],
                                         vb[0:p_in, h:h + 512],
                                         start=True, stop=False)
                        nc.tensor.matmul(sup[:, h:h + 512], a_sb[0:p_in, :],
                                         vb[0:p_in, h + 2:h + 514],
                                         start=False, stop=False)
                        nc.tensor.matmul(sup[:, h:h + 512], a_sb[0:p_in, :],
                                         ub[0:p_in, h + 4:h + 516],
                                         start=False, stop=True)

                    # |Su| to SBUF (ScalarE Abs on the PSUM read)
                    asu = asu_pool.tile([128, W], bf16)
                    nc.scalar.activation(out=asu[:], in_=sup[:], func=Act.Abs)

                    # le = (|Su| < V-0.5) * loss, accumulated per partition.
                    # main covers interior cols; image-edge cols/rows redone
                    # with their own thresholds into separate accumulators.
                    scr = scr_pool.tile([128, W], bf16)
                    nc.vector.scalar_tensor_tensor(
                        out=scr[0:p_in, 2:W - 2], in0=asu[0:p_in, 2:W - 2],
                        scalar=thrm_sb[0:p_in, 0:1], in1=loss[0:p_in, 2:W - 2],
                        op0=Alu.is_lt, op1=Alu.mult,
                        accum_out=stats[0:p_in, idx:idx + 1])
                    # cols {0, W-1}: vw=3; cols {1, W-2}: vw=4 (strided pairs)
                    for coff, cstride, thr_sb, base in (
                            (0, W - 1, thra_sb, 40), (1, W - 3, thrb_sb, 80)):
                        asu_e = bass.AP(
                            tensor=asu[:].tensor,
                            offset=asu[:].offset + coff,
                            ap=[[asu[:].ap[0][0], p_in], [cstride, 2]])
                        loss_e = bass.AP(
                            tensor=loss[:].tensor,
                            offset=loss[:].offset + coff,
                            ap=[[loss[:].ap[0][0], p_in], [cstride, 2]])
                        scr_e = bass.AP(
                            tensor=scr[:].tensor,
                            offset=scr[:].offset + coff,
                            ap=[[scr[:].ap[0][0], p_in], [cstride, 2]])
                        nc.vector.scalar_tensor_tensor(
                            out=scr_e, in0=asu_e, scalar=thr_sb[0:p_in, 0:1],
                            in1=loss_e, op0=Alu.is_lt, op1=Alu.mult,
                            accum_out=stats[0:p_in, base + idx:base + idx + 1])
                    # image top/bottom rows (interior cols, vw=5)
                    if t == 0 or t == NT - 1:
                        rr = 2 if t == 0 else 32
                        nc.vector.scalar_tensor_tensor(
                            out=scr[0:rr, 2:W - 2], in0=asu[0:rr, 2:W - 2],
                            scalar=thrr_sb[0:rr, 0:1], in1=loss[0:rr, 2:W - 2],
                            op0=Alu.is_lt, op1=Alu.mult,
                            accum_out=stats[0:rr, 120 + rowidx:121 + rowidx])
                        rowidx = (rowidx + 1) % 8
                    idx += 1

            red = sing.tile([128, 2], f32, tag="red")
            nc.vector.reduce_sum(out=red[:, 0:1], in_=stats[:, 160:196],
                                 axis=mybir.AxisListType.X)
            nc.vector.reduce_sum(out=red[:, 1:2], in_=stats[:, 0:128],
                                 axis=mybir.AxisListType.X)
            nc.sync.dma_start(out=out_d[:], in_=red[:])

    _split_multi_waits(nc, mybir)
    return nc


def _get_nc():
    if "nc" not in _CACHED:
        _CACHED["nc"] = _build_nc()
    return _CACHED["nc"]


def run(pred: np.ndarray, target: np.ndarray, trace: bool = False):
    """Returns (result_scalar, BassKernelResults)."""
    from concourse import bass_utils

    nc = _get_nc()
    statics = _statics()
    pred = np.ascontiguousarray(np.asarray(pred).reshape(B * H, W),
                                dtype=np.float32)
    target = np.ascontiguousarray(np.asarray(target).reshape(B * H, W),
                                  dtype=np.float32)
    in_maps = []
    for c in range(NCORES):
        m = dict(statics)
        m["pred"] = pred[c * ROWS:(c + 1) * ROWS]
        m["target"] = target[c * ROWS:(c + 1) * ROWS]
        in_maps.append(m)
    res = bass_utils.run_bass_kernel_spmd(
        nc, in_maps, core_ids=list(range(NCORES)), trace=trace)
    s_loss = 0.0
    s_le = 0.0
    for r in res.results:
        o = r["out"].astype(np.float64)
        s_loss += o[:, 0].sum()
        s_le += o[:, 1].sum()
    val = np.float32((s_loss - 0.9 * s_le) / N_TOT)
    return np.asarray(val, dtype=np.float32), res


def kernel(pred: np.ndarray, target: np.ndarray) -> np.ndarray:
    val, _ = run(pred, target, trace=False)
    return val


if __name__ == "__main__":
    rng = np.random.default_rng(0)
    p = rng.standard_normal((B, 1, H, W)).astype(np.float32)
    t = rng.integers(0, 2, (B, 1, H, W)).astype(np.float32)
    print(kernel(pred=p, target=t))


# revision 20
# speedup vs baseline: 1.0950x; 1.0950x over previous
"""BBox-aware BCE loss kernel for Trainium2 (8 NeuronCores, data parallel).

Math (exact reformulation of the reference):
  loss = softplus(pred) - pred*target = softplus((1-2t)*pred)   for t in {0,1}
  u = 1-2t in {+1 (t=0), -1 (t=1)}
  Su(i,j) = sum of u over the clipped 5x5 window = (#zeros - #ones)
  edge pixel  <=>  window is mixed  <=>  |Su| < V   (V = clipped window size)
  result = (sum(loss) - 0.9*sum(loss*edge)) / N   (equals the reference in
  both branches of its global `cond`: no edges anywhere => sum(loss*edge)=0).

Device pipeline per core (4 samples, 9 overlapping 128-row tiles each so the
5-tap column window never crosses a tile boundary; owned rows exclude the
2-row overlap):
  GpSimd: casting DMAs (f32 HBM -> bf16 SBUF) for pred/target; first
          row-window pair-add v[j] = u[j]+u[j+1]
  VectorE: u = 1-2t; s = p*u; fused (|Su|<V)*loss with accumulated sums
           (main + image-edge strips with their own thresholds)
  ScalarE: g = exp(s); loss = ln(g*own + 1) (own=0 kills overlap rows);
           |Su| via Abs on the PSUM->SBUF copy
  TensorE: column 5-window sum via 3 shifted accumulating band matmuls
           (Su = A@(v(-2) + v(0) + u(+2))); sum(loss) via ones-matmuls
           accumulated in PSUM over all tiles
Host: float64 reduction of per-core partials.
"""

import sys

import numpy as np

sys.path.insert(0, "/opt/trn_rl_repo")

import ml_dtypes

B, H, W = 32, 1024, 1024
NCORES = 8
SPC = B // NCORES  # samples per core
ROWS = SPC * H
N_TOT = float(B * H * W)

# per-sample tiling: (input_row_start, input_rows, owned_lo, owned_hi)
TILES = [(0, 128, 0, 126)]
for t in range(1, 8):
    TILES.append((124 * t, 128, 2, 126))
TILES.append((992, 32, 2, 32))
NT = len(TILES)  # 9
NTILES = SPC * NT  # 36

BF16 = ml_dtypes.bfloat16


def _band(k_rows: int, m_lo: int, m_hi: int) -> np.ndarray:
    a = np.zeros((k_rows, 128), dtype=np.float32)
    for k in range(k_rows):
        for m in range(m_lo, m_hi):
            if abs(k - m) <= 2:
                a[k, m] = 1.0
    return a.astype(BF16)


def _vh_of(tile_idx: int) -> np.ndarray:
    """clipped column-window size per in-tile row."""
    in0, p_in, _, _ = TILES[tile_idx]
    vh = np.full(128, 5.0, dtype=np.float32)
    for k in range(p_in):
        img = in0 + k
        vh[k] = min(img, 2) + min(H - 1 - img, 2) + 1
    return vh


def _owned(tile_idx: int) -> np.ndarray:
    _, _, o0, o1 = TILES[tile_idx]
    m = np.zeros(128, dtype=np.float32)
    m[o0:o1] = 1.0
    return m


def _statics() -> dict[str, np.ndarray]:
    s = {}
    s["a_top"] = _band(128, 0, 126)
    s["a_mid"] = _band(128, 2, 126)
    s["a_last"] = _band(32, 2, 32)
    for nm, ti in (("top", 0), ("mid", 1), ("last", 8)):
        vh = _vh_of(ti)
        own = _owned(ti)
        edge_row = (vh < 5.0) & (own > 0)  # image top/bottom rows (owned)
        s[f"ones_{nm}"] = own.astype(BF16).reshape(128, 1)
        s[f"own_{nm}"] = own.reshape(128, 1).astype(np.float32)
        # exp bias: -100 on non-owned rows => g ~ 0 => loss = ln(1) = 0
        s[f"kill_{nm}"] = (100.0 * (own - 1.0)).reshape(128, 1).astype(
            np.float32)
        # Per-partition |Su| thresholds; -1 disables a row (never an edge,
        # contributes 0) so the four accumulation regions are exactly
        # disjoint and overlap rows are excluded everywhere.
        # main: interior cols, vh=5 owned rows only
        thrm = np.where(own > 0, 24.5, -1.0)
        thrm = np.where(edge_row, -1.0, thrm)
        s[f"thrm_{nm}"] = thrm.reshape(128, 1).astype(np.float32)
        # edge cols {0,W-1}: vw=3; {1,W-2}: vw=4 (all owned rows)
        s[f"thra_{nm}"] = np.where(own > 0, vh * 3.0 - 0.5, -1.0).reshape(
            128, 1).astype(np.float32)
        s[f"thrb_{nm}"] = np.where(own > 0, vh * 4.0 - 0.5, -1.0).reshape(
            128, 1).astype(np.float32)
        # image top/bottom rows, interior cols (vw=5)
        s[f"thrr_{nm}"] = np.where(edge_row, vh * 5.0 - 0.5, -1.0).reshape(
            128, 1).astype(np.float32)
    return s


_CACHED = {}


def _split_multi_waits(nc, mybir):
    """This walrus's core_v3 codegen allows only one sem-wait per
    instruction; peel extra waits onto same-engine NOPs placed just before."""
    skip = (mybir.InstEventSemaphore,)
    k = 0
    for fn in nc.m.functions:
        for blk in fn.blocks:
            out = []
            for inst in blk.instructions:
                si = inst.sync_info
                if (si is not None and len(si.on_wait) > 1
                        and not isinstance(inst, skip)):
                    waits = list(si.on_wait)
                    for w in waits[:-1]:
                        k += 1
                        nop = mybir.InstNoOp(name=f"wsplit-{k}", ins=[], outs=[])
                        nop.engine = inst.engine
                        nop.sync_info = mybir.SyncInfo(on_wait=[w], on_update=[])
                        out.append(nop)
                    inst.sync_info = mybir.SyncInfo(
                        on_wait=[waits[-1]], on_update=list(si.on_update))
                out.append(inst)
            blk.instructions = out


def _build_nc():
    import concourse.bass as bass
    import concourse.mybir as mybir
    import concourse.tile as tile

    f32 = mybir.dt.float32
    bf16 = mybir.dt.bfloat16
    Act = mybir.ActivationFunctionType
    Alu = mybir.AluOpType

    nc = bass.Bass("TRN2", target_bir_lowering=False, debug=False,
                   num_devices=NCORES, num_swdge_queues=4)

    pred_d = nc.dram_tensor("pred", [ROWS, W], f32, kind="ExternalInput").ap()
    tgt_d = nc.dram_tensor("target", [ROWS, W], f32, kind="ExternalInput").ap()
    sd = {}
    statics = _statics()
    for nm, arr in statics.items():
        dt = bf16 if arr.dtype == BF16 else f32
        sd[nm] = nc.dram_tensor(nm, list(arr.shape), dt,
                                kind="ExternalInput").ap()
    out_d = nc.dram_tensor("out", [128, 40], f32, kind="ExternalOutput").ap()

    WP = W + 4  # padded width for the 5-tap row window

    with tile.TileContext(nc) as tc:
        with (
            tc.tile_pool(name="sing", bufs=1) as sing,
            tc.tile_pool(name="tb", bufs=4) as tb_pool,
            tc.tile_pool(name="pb", bufs=4) as pb_pool,
            tc.tile_pool(name="g", bufs=3) as g_pool,
            tc.tile_pool(name="loss", bufs=4) as loss_pool,
            tc.tile_pool(name="asu", bufs=3) as asu_pool,
            tc.tile_pool(name="scr", bufs=3) as scr_pool,
            tc.tile_pool(name="psum", bufs=3, space="PSUM") as psum_pool,
        ):
            # ---- statics in SBUF ----
            sb = {}
            for nm, arr in statics.items():
                dt = bf16 if arr.dtype == BF16 else f32
                sb[nm] = sing.tile(list(arr.shape), dt, tag=nm, name=nm)
                nc.sync.dma_start(out=sb[nm][:], in_=sd[nm][:])

            def per_tile(t):
                nm = "top" if t == 0 else ("last" if t == NT - 1 else "mid")
                return (sb[f"a_{nm}"], sb[f"thrm_{nm}"],
                        sb[f"thra_{nm}"], sb[f"thrb_{nm}"], sb[f"thrr_{nm}"])

            # stats columns: [0:36) main le, [40:76) colA, [80:116) colB,
            # [120:128) row strips, [160:196) loss sums
            stats = sing.tile([128, 224], f32, tag="stats")
            nc.vector.memset(stats[:], 0.0)

            # padded ring buffers (pads zeroed once, never rewritten)
            u_bufs = [sing.tile([128, WP], bf16, tag=f"ub{i}", name=f"ub{i}")
                      for i in range(4)]
            v_bufs = [sing.tile([128, WP], bf16, tag=f"vb{i}", name=f"vb{i}")
                      for i in range(4)]
            for bb in u_bufs + v_bufs:
                nc.vector.memset(bb[:, 0:2], 0.0)
                nc.vector.memset(bb[:, W + 2:WP], 0.0)

            idx = 0
            rowidx = 0
            for smp in range(SPC):
                for t in range(NT):
                    in0, p_in, o0, o1 = TILES[t]
                    r0 = smp * H + in0
                    a_sb, thrm_sb, thra_sb, thrb_sb, thrr_sb = per_tile(t)

                    # casting DMAs: f32 HBM -> bf16 SBUF
                    tb = tb_pool.tile([128, W], bf16)
                    nc.gpsimd.dma_start(out=tb[0:p_in],
                                        in_=tgt_d[r0:r0 + p_in, :])
                    pb = pb_pool.tile([128, W], bf16)
                    nc.gpsimd.dma_start(out=pb[0:p_in],
                                        in_=pred_d[r0:r0 + p_in, :])

                    # u = 1 - 2t into padded buffer center
                    ub = u_bufs[idx % 4]
                    nc.vector.tensor_scalar(
                        out=ub[0:p_in, 2:2 + W], in0=tb[0:p_in],
                        scalar1=-2.0, scalar2=1.0, op0=Alu.mult, op1=Alu.add)

                    # s = p*u (in place over pb)
                    nc.vector.tensor_mul(out=pb[0:p_in], in0=pb[0:p_in],
                                         in1=ub[0:p_in, 2:2 + W])

                    # loss = ln(exp(s)+1); ln accumulates per-partition
                    # sums (overlap rows excluded host-side per tile).
                    g = g_pool.tile([128, W], bf16)
                    nc.scalar.activation(out=g[0:p_in], in_=pb[0:p_in],
                                         func=Act.Exp)
                    loss = loss_pool.tile([128, W], f32)
                    nc.scalar.activation(out=loss[0:p_in], in_=g[0:p_in],
                                         func=Act.Ln, bias=1.0,
                                         accum_out=stats[0:p_in,
                                                         160 + idx:161 + idx])

                    # v[j] = u[j] + u[j+1] (first half of the row 5-window)
                    vb = v_bufs[idx % 4]
                    nc.gpsimd.tensor_add(out=vb[0:p_in, 0:1026],
                                         in0=ub[0:p_in, 0:1026],
                                         in1=ub[0:p_in, 1:1027])

                    # Su = A @ (v(-2) + v(0) + u(+2)): 3 accumulating matmuls
                    sup = psum_pool.tile([128, W], f32)
                    for h in (0, 512):
                        nc.tensor.matmul(sup[:, h:h + 512], a_sb[0:p_in, :],
                                         vb[0:p_in, h:h + 512],
                                         start=True, stop=False)
                        nc.tensor.matmul(sup[:, h:h + 512], a_sb[0:p_in, :],
                                         vb[0:p_in, h + 2:h + 514],
                                         start=False, stop=False)
                        nc.tensor.matmul(sup[:, h:h + 512], a_sb[0:p_in, :],
                                         ub[0:p_in, h + 4:h + 516],
                                         start=False, stop=True)

                    # |Su| to SBUF (ScalarE Abs on the PSUM read)
                    asu = asu_pool.tile([128, W], bf16)
                    nc.scalar.activation(out=asu[:], in_=sup[:], func=Act.Abs)

                    # le = (|Su| < V-0.5) * loss, accumulated per partition.
                    # main covers interior cols; image-edge cols/rows redone
                    # with their own thresholds into separate accumulators.
                    scr = scr_pool.tile([128, W], bf16)
                    nc.vector.scalar_tensor_tensor(
                        out=scr[0:p_in, 2:W - 2], in0=asu[0:p_in, 2:W - 2],
                        scalar=thrm_sb[0:p_in, 0:1], in1=loss[0:p_in, 2:W - 2],
                        op0=Alu.is_lt, op1=Alu.mult,
                        accum_out=stats[0:p_in, idx:idx + 1])
                    # cols {0, W-1}: vw=3; cols {1, W-2}: vw=4 (strided pairs)
                    for coff, cstride, thr_sb, base in (
                            (0, W - 1, thra_sb, 40), (1, W - 3, thrb_sb, 80)):
                        asu_e = bass.AP(
                            tensor=asu[:].tensor,
                            offset=asu[:].offset + coff,
                            ap=[[asu[:].ap[0][0], p_in], [cstride, 2]])
                        loss_e = bass.AP(
                            tensor=loss[:].tensor,
                            offset=loss[:].offset + coff,
                            ap=[[loss[:].ap[0][0], p_in], [cstride, 2]])
                        scr_e = bass.AP(
                            tensor=scr[:].tensor,
                            offset=scr[:].offset + coff,
                            ap=[[scr[:].ap[0][0], p_in], [cstride, 2]])
                        nc.vector.scalar_tensor_tensor(
                            out=scr_e, in0=asu_e, scalar=thr_sb[0:p_in, 0:1],
                            in1=loss_e, op0=Alu.is_lt, op1=Alu.mult,
                            accum_out=stats[0:p_in, base + idx:base + idx + 1])
                    # image top/bottom rows (interior cols, vw=5)
                    if t == 0 or t == NT - 1:
                        rr = 2 if t == 0 else 32
                        nc.vector.scalar_tensor_tensor(
                            out=scr[0:rr, 2:W - 2], in0=asu[0:rr, 2:W - 2],
                            scalar=thrr_sb[0:rr, 0:1], in1=loss[0:rr, 2:W - 2],
                            op0=Alu.is_lt, op1=Alu.mult,
                            accum_out=stats[0:rr, 120 + rowidx:121 + rowidx])
                        rowidx = (rowidx + 1) % 8
                    idx += 1

            red = sing.tile([128, 40], f32, tag="red")
            nc.vector.memset(red[:, 1:4], 0.0)
            nc.vector.reduce_sum(out=red[:, 0:1], in_=stats[:, 0:128],
                                 axis=mybir.AxisListType.X)
            nc.vector.tensor_copy(out=red[:, 4:40], in_=stats[:, 160:196])
            nc.sync.dma_start(out=out_d[:], in_=red[:])

    _split_multi_waits(nc, mybir)
    return nc


def _get_nc():
    if "nc" not in _CACHED:
        _CACHED["nc"] = _build_nc()
    return _CACHED["nc"]


def run(pred: np.ndarray, target: np.ndarray, trace: bool = False):
    """Returns (result_scalar, BassKernelResults)."""
    from concourse import bass_utils

    nc = _get_nc()
    statics = _statics()
    pred = np.ascontiguousarray(np.asarray(pred).reshape(B * H, W),
                                dtype=np.float32)
    target = np.ascontiguousarray(np.asarray(target).reshape(B * H, W),
                                  dtype=np.float32)
    in_maps = []
    for c in range(NCORES):
        m = dict(statics)
        m["pred"] = pred[c * ROWS:(c + 1) * ROWS]
        m["target"] = target[c * ROWS:(c + 1) * ROWS]
        in_maps.append(m)
    res = bass_utils.run_bass_kernel_spmd(
        nc, in_maps, core_ids=list(range(NCORES)), trace=trace)
    s_loss = 0.0
    s_le = 0.0
    for r in res.results:
        o = r["out"].astype(np.float64)
        s_le += o[:, 0].sum()
        for ti in range(NTILES):
            _, _, o0, o1 = TILES[ti % NT]
            s_loss += o[o0:o1, 4 + ti].sum()
    val = np.float32((s_loss - 0.9 * s_le) / N_TOT)
    return np.asarray(val, dtype=np.float32), res


def kernel(pred: np.ndarray, target: np.ndarray) -> np.ndarray:
    val, _ = run(pred, target, trace=False)
    return val


if __name__ == "__main__":
    rng = np.random.default_rng(0)
    p = rng.standard_normal((B, 1, H, W)).astype(np.float32)
    t = rng.integers(0, 2, (B, 1, H, W)).astype(np.float32)
    print(kernel(pred=p, target=t))


# revision 22
# speedup vs baseline: 1.2481x; 1.1399x over previous
"""BBox-aware BCE loss kernel for Trainium2 (8 NeuronCores, data parallel).

Math (exact reformulation of the reference):
  loss = softplus(pred) - pred*target = softplus((1-2t)*pred)   for t in {0,1}
  u = 1-2t in {+1 (t=0), -1 (t=1)}
  Su(i,j) = sum of u over the clipped 5x5 window = (#zeros - #ones)
  edge pixel  <=>  window is mixed  <=>  |Su| < V   (V = clipped window size)
  result = (sum(loss) - 0.9*sum(loss*edge)) / N   (equals the reference in
  both branches of its global `cond`: no edges anywhere => sum(loss*edge)=0).

Device pipeline per core (4 samples, 9 overlapping 128-row tiles each so the
5-tap column window never crosses a tile boundary; owned rows exclude the
2-row overlap):
  GpSimd: casting DMAs (f32 HBM -> bf16 SBUF) for pred/target; first
          row-window pair-add v[j] = u[j]+u[j+1]
  VectorE: u = 1-2t; s = p*u; fused (|Su|<V)*loss with accumulated sums
           (main + image-edge strips with their own thresholds)
  ScalarE: g = exp(s); loss = ln(g*own + 1) (own=0 kills overlap rows);
           |Su| via Abs on the PSUM->SBUF copy
  TensorE: column 5-window sum via 3 shifted accumulating band matmuls
           (Su = A@(v(-2) + v(0) + u(+2))); sum(loss) via ones-matmuls
           accumulated in PSUM over all tiles
Host: float64 reduction of per-core partials.
"""

import sys

import numpy as np

sys.path.insert(0, "/opt/trn_rl_repo")

import ml_dtypes


def _setup_act_tables() -> None:
    """Create a patched ACT-table dir exposing softplus (the act2 slot of the
    stock softplus_and_others set) and point BASS_ACT_ROOT_JSON_PATH at it."""
    import json
    import os
    import tempfile
    from pathlib import Path

    if os.environ.get("BASS_ACT_ROOT_JSON_PATH"):
        return
    import neuronxcc

    stock = Path(neuronxcc.__file__).parent / "pwp" / "pwp_bin_trainium"
    if not stock.exists():
        return
    dst = Path(tempfile.mkdtemp(prefix="act_tables_"))
    for f in stock.iterdir():
        if f.name not in ("act_info.json", "softplus_and_others.json"):
            (dst / f.name).symlink_to(f)
    info = json.loads((stock / "act_info.json").read_text())
    for s in info["act_func_sets"]:
        if s["name"] == "softplus_and_others":
            s["act"]["softplus"] = s["act"].get("act2", 1)
    (dst / "act_info.json").write_text(json.dumps(info))
    prof = json.loads((stock / "softplus_and_others.json").read_text())
    for key in ("func_to_bkt_start_idx", "func_to_ctl_start_idx",
                "func_exp_to_bkt_start_idx", "func_exp_to_ctl_start_idx"):
        if key in prof and "act2" in prof[key]:
            prof[key]["softplus"] = prof[key]["act2"]
    (dst / "softplus_and_others.json").write_text(json.dumps(prof))
    os.environ["BASS_ACT_ROOT_JSON_PATH"] = str(dst / "act_info.json")



B, H, W = 32, 1024, 1024
NCORES = 8
SPC = B // NCORES  # samples per core
ROWS = SPC * H
N_TOT = float(B * H * W)

# per-sample tiling: (input_row_start, input_rows, owned_lo, owned_hi)
TILES = [(0, 128, 0, 126)]
for t in range(1, 8):
    TILES.append((124 * t, 128, 2, 126))
TILES.append((992, 32, 2, 32))
NT = len(TILES)  # 9
NTILES = SPC * NT  # 36

BF16 = ml_dtypes.bfloat16


def _band(k_rows: int, m_lo: int, m_hi: int) -> np.ndarray:
    a = np.zeros((k_rows, 128), dtype=np.float32)
    for k in range(k_rows):
        for m in range(m_lo, m_hi):
            if abs(k - m) <= 2:
                a[k, m] = 1.0
    return a.astype(BF16)


def _vh_of(tile_idx: int) -> np.ndarray:
    """clipped column-window size per in-tile row."""
    in0, p_in, _, _ = TILES[tile_idx]
    vh = np.full(128, 5.0, dtype=np.float32)
    for k in range(p_in):
        img = in0 + k
        vh[k] = min(img, 2) + min(H - 1 - img, 2) + 1
    return vh


def _owned(tile_idx: int) -> np.ndarray:
    _, _, o0, o1 = TILES[tile_idx]
    m = np.zeros(128, dtype=np.float32)
    m[o0:o1] = 1.0
    return m


def _statics() -> dict[str, np.ndarray]:
    s = {}
    s["a_top"] = _band(128, 0, 126)
    s["a_mid"] = _band(128, 2, 126)
    s["a_last"] = _band(32, 2, 32)
    for nm, ti in (("top", 0), ("mid", 1), ("last", 8)):
        vh = _vh_of(ti)
        own = _owned(ti)
        edge_row = (vh < 5.0) & (own > 0)  # image top/bottom rows (owned)
        s[f"ones_{nm}"] = own.astype(BF16).reshape(128, 1)
        s[f"own_{nm}"] = own.reshape(128, 1).astype(np.float32)
        # exp bias: -100 on non-owned rows => g ~ 0 => loss = ln(1) = 0
        s[f"kill_{nm}"] = (100.0 * (own - 1.0)).reshape(128, 1).astype(
            np.float32)
        # Per-partition |Su| thresholds; -1 disables a row (never an edge,
        # contributes 0) so the four accumulation regions are exactly
        # disjoint and overlap rows are excluded everywhere.
        # main: interior cols, vh=5 owned rows only
        thrm = np.where(own > 0, 24.5, -1.0)
        thrm = np.where(edge_row, -1.0, thrm)
        s[f"thrm_{nm}"] = thrm.reshape(128, 1).astype(np.float32)
        # edge cols {0,W-1}: vw=3; {1,W-2}: vw=4 (all owned rows)
        s[f"thra_{nm}"] = np.where(own > 0, vh * 3.0 - 0.5, -1.0).reshape(
            128, 1).astype(np.float32)
        s[f"thrb_{nm}"] = np.where(own > 0, vh * 4.0 - 0.5, -1.0).reshape(
            128, 1).astype(np.float32)
        # image top/bottom rows, interior cols (vw=5)
        s[f"thrr_{nm}"] = np.where(edge_row, vh * 5.0 - 0.5, -1.0).reshape(
            128, 1).astype(np.float32)
    return s


_CACHED = {}


def _split_multi_waits(nc, mybir):
    """This walrus's core_v3 codegen allows only one sem-wait per
    instruction; peel extra waits onto same-engine NOPs placed just before."""
    skip = (mybir.InstEventSemaphore,)
    k = 0
    for fn in nc.m.functions:
        for blk in fn.blocks:
            out = []
            for inst in blk.instructions:
                si = inst.sync_info
                if (si is not None and len(si.on_wait) > 1
                        and not isinstance(inst, skip)):
                    waits = list(si.on_wait)
                    for w in waits[:-1]:
                        k += 1
                        nop = mybir.InstNoOp(name=f"wsplit-{k}", ins=[], outs=[])
                        nop.engine = inst.engine
                        nop.sync_info = mybir.SyncInfo(on_wait=[w], on_update=[])
                        out.append(nop)
                    inst.sync_info = mybir.SyncInfo(
                        on_wait=[waits[-1]], on_update=list(si.on_update))
                out.append(inst)
            blk.instructions = out


def _build_nc():
    import concourse.bass as bass
    import concourse.mybir as mybir
    import concourse.tile as tile

    f32 = mybir.dt.float32
    bf16 = mybir.dt.bfloat16
    Act = mybir.ActivationFunctionType
    Alu = mybir.AluOpType

    nc = bass.Bass("TRN2", target_bir_lowering=False, debug=False,
                   num_devices=NCORES, num_swdge_queues=4)

    pred_d = nc.dram_tensor("pred", [ROWS, W], f32, kind="ExternalInput").ap()
    tgt_d = nc.dram_tensor("target", [ROWS, W], f32, kind="ExternalInput").ap()
    sd = {}
    statics = _statics()
    for nm, arr in statics.items():
        dt = bf16 if arr.dtype == BF16 else f32
        sd[nm] = nc.dram_tensor(nm, list(arr.shape), dt,
                                kind="ExternalInput").ap()
    out_d = nc.dram_tensor("out", [128, 40], f32, kind="ExternalOutput").ap()

    WP = W + 4  # padded width for the 5-tap row window

    with tile.TileContext(nc) as tc:
        with (
            tc.tile_pool(name="sing", bufs=1) as sing,
            tc.tile_pool(name="tb", bufs=4) as tb_pool,
            tc.tile_pool(name="pb", bufs=4) as pb_pool,
            tc.tile_pool(name="g", bufs=3) as g_pool,
            tc.tile_pool(name="loss", bufs=4) as loss_pool,
            tc.tile_pool(name="asu", bufs=3) as asu_pool,
            tc.tile_pool(name="scr", bufs=3) as scr_pool,
            tc.tile_pool(name="psum", bufs=3, space="PSUM") as psum_pool,
        ):
            # ---- statics in SBUF ----
            sb = {}
            for nm, arr in statics.items():
                dt = bf16 if arr.dtype == BF16 else f32
                sb[nm] = sing.tile(list(arr.shape), dt, tag=nm, name=nm)
                nc.sync.dma_start(out=sb[nm][:], in_=sd[nm][:])

            def per_tile(t):
                nm = "top" if t == 0 else ("last" if t == NT - 1 else "mid")
                return (sb[f"a_{nm}"], sb[f"thrm_{nm}"],
                        sb[f"thra_{nm}"], sb[f"thrb_{nm}"], sb[f"thrr_{nm}"])

            # stats columns: [0:36) main le, [40:76) colA, [80:116) colB,
            # [120:128) row strips, [160:196) loss sums
            stats = sing.tile([128, 224], f32, tag="stats")
            nc.vector.memset(stats[:], 0.0)

            # padded ring buffers (pads zeroed once, never rewritten)
            u_bufs = [sing.tile([128, WP], bf16, tag=f"ub{i}", name=f"ub{i}")
                      for i in range(4)]
            for bb in u_bufs:
                nc.vector.memset(bb[:, 0:2], 0.0)
                nc.vector.memset(bb[:, W + 2:WP], 0.0)

            idx = 0
            rowidx = 0
            for smp in range(SPC):
                for t in range(NT):
                    in0, p_in, o0, o1 = TILES[t]
                    r0 = smp * H + in0
                    a_sb, thrm_sb, thra_sb, thrb_sb, thrr_sb = per_tile(t)

                    # casting DMAs: f32 HBM -> bf16 SBUF
                    tb = tb_pool.tile([128, W], bf16)
                    nc.gpsimd.dma_start(out=tb[0:p_in],
                                        in_=tgt_d[r0:r0 + p_in, :])
                    pb = pb_pool.tile([128, W], bf16)
                    nc.gpsimd.dma_start(out=pb[0:p_in],
                                        in_=pred_d[r0:r0 + p_in, :])

                    # u = 1 - 2t into padded buffer center
                    ub = u_bufs[idx % 4]
                    nc.vector.tensor_scalar(
                        out=ub[0:p_in, 2:2 + W], in0=tb[0:p_in],
                        scalar1=-2.0, scalar2=1.0, op0=Alu.mult, op1=Alu.add)

                    # s = p*u (in place over pb)
                    nc.vector.tensor_mul(out=pb[0:p_in], in0=pb[0:p_in],
                                         in1=ub[0:p_in, 2:2 + W])

                    # loss = ln(exp(s)+1); ln accumulates per-partition
                    # sums (overlap rows excluded host-side per tile).
                    g = g_pool.tile([128, W], bf16)
                    nc.scalar.activation(out=g[0:p_in], in_=pb[0:p_in],
                                         func=Act.Exp)
                    loss = loss_pool.tile([128, W], f32)
                    nc.scalar.activation(out=loss[0:p_in], in_=g[0:p_in],
                                         func=Act.Ln, bias=1.0,
                                         accum_out=stats[0:p_in,
                                                         160 + idx:161 + idx])

                    # Su = A @ sum_d u(d), d=-2..2: 5 shifted accumulating
                    # matmuls (same stationary weights)
                    sup = psum_pool.tile([128, W], f32)
                    for h in (0, 512):
                        for dd in range(5):
                            nc.tensor.matmul(sup[:, h:h + 512],
                                             a_sb[0:p_in, :],
                                             ub[0:p_in, h + dd:h + dd + 512],
                                             start=(dd == 0), stop=(dd == 4))

                    # |Su| to SBUF (ScalarE Abs on the PSUM read)
                    asu = asu_pool.tile([128, W], bf16)
                    nc.scalar.activation(out=asu[:], in_=sup[:], func=Act.Abs)

                    # le = (|Su| < V-0.5) * loss, accumulated per partition.
                    # main covers interior cols; image-edge cols/rows redone
                    # with their own thresholds into separate accumulators.
                    scr = scr_pool.tile([128, W], bf16)
                    nc.vector.scalar_tensor_tensor(
                        out=scr[0:p_in, 2:W - 2], in0=asu[0:p_in, 2:W - 2],
                        scalar=thrm_sb[0:p_in, 0:1], in1=loss[0:p_in, 2:W - 2],
                        op0=Alu.is_lt, op1=Alu.mult,
                        accum_out=stats[0:p_in, idx:idx + 1])
                    # cols {0, W-1}: vw=3; cols {1, W-2}: vw=4 (strided pairs)
                    for coff, cstride, thr_sb, base in (
                            (0, W - 1, thra_sb, 40), (1, W - 3, thrb_sb, 80)):
                        asu_e = bass.AP(
                            tensor=asu[:].tensor,
                            offset=asu[:].offset + coff,
                            ap=[[asu[:].ap[0][0], p_in], [cstride, 2]])
                        loss_e = bass.AP(
                            tensor=loss[:].tensor,
                            offset=loss[:].offset + coff,
                            ap=[[loss[:].ap[0][0], p_in], [cstride, 2]])
                        scr_e = bass.AP(
                            tensor=scr[:].tensor,
                            offset=scr[:].offset + coff,
                            ap=[[scr[:].ap[0][0], p_in], [cstride, 2]])
                        nc.vector.scalar_tensor_tensor(
                            out=scr_e, in0=asu_e, scalar=thr_sb[0:p_in, 0:1],
                            in1=loss_e, op0=Alu.is_lt, op1=Alu.mult,
                            accum_out=stats[0:p_in, base + idx:base + idx + 1])
                    # image top/bottom rows (interior cols, vw=5)
                    if t == 0 or t == NT - 1:
                        rr = 2 if t == 0 else 32
                        nc.vector.scalar_tensor_tensor(
                            out=scr[0:rr, 2:W - 2], in0=asu[0:rr, 2:W - 2],
                            scalar=thrr_sb[0:rr, 0:1], in1=loss[0:rr, 2:W - 2],
                            op0=Alu.is_lt, op1=Alu.mult,
                            accum_out=stats[0:rr, 120 + rowidx:121 + rowidx])
                        rowidx = (rowidx + 1) % 8
                    idx += 1

            red = sing.tile([128, 40], f32, tag="red")
            nc.vector.memset(red[:, 1:4], 0.0)
            nc.vector.reduce_sum(out=red[:, 0:1], in_=stats[:, 0:128],
                                 axis=mybir.AxisListType.X)
            nc.vector.tensor_copy(out=red[:, 4:40], in_=stats[:, 160:196])
            nc.sync.dma_start(out=out_d[:], in_=red[:])

    _split_multi_waits(nc, mybir)
    return nc


def _get_nc():
    if "nc" not in _CACHED:
        _CACHED["nc"] = _build_nc()
    return _CACHED["nc"]


def run(pred: np.ndarray, target: np.ndarray, trace: bool = False):
    """Returns (result_scalar, BassKernelResults)."""
    from concourse import bass_utils

    nc = _get_nc()
    statics = _statics()
    pred = np.ascontiguousarray(np.asarray(pred).reshape(B * H, W),
                                dtype=np.float32)
    target = np.ascontiguousarray(np.asarray(target).reshape(B * H, W),
                                  dtype=np.float32)
    in_maps = []
    for c in range(NCORES):
        m = dict(statics)
        m["pred"] = pred[c * ROWS:(c + 1) * ROWS]
        m["target"] = target[c * ROWS:(c + 1) * ROWS]
        in_maps.append(m)
    res = bass_utils.run_bass_kernel_spmd(
        nc, in_maps, core_ids=list(range(NCORES)), trace=trace)
    s_loss = 0.0
    s_le = 0.0
    for r in res.results:
        o = r["out"].astype(np.float64)
        s_le += o[:, 0].sum()
        for ti in range(NTILES):
            _, _, o0, o1 = TILES[ti % NT]
            s_loss += o[o0:o1, 4 + ti].sum()
    val = np.float32((s_loss - 0.9 * s_le) / N_TOT)
    return np.asarray(val, dtype=np.float32), res


def kernel(pred: np.ndarray, target: np.ndarray) -> np.ndarray:
    val, _ = run(pred, target, trace=False)
    return val


if __name__ == "__main__":
    rng = np.random.default_rng(0)
    p = rng.standard_normal((B, 1, H, W)).astype(np.float32)
    t = rng.integers(0, 2, (B, 1, H, W)).astype(np.float32)
    print(kernel(pred=p, target=t))
